# revision 32
# baseline (speedup 1.0000x reference)
"""Trainium2 Bass kernel for GQA multi-head attention block (nn_MHA_68831145886222).

Computation (reference):
  qkv = x @ w_qkv.T ; split q[32 heads],k[8],v[8] (HD=128)
  q,k = rmsnorm(head_dim) -> rope(interleaved, theta=1e6)
  out = causal GQA attention (4 q heads per kv head)
  y   = (attn out) @ w_out.T

Sharding: tensor-parallel by kv-head group. Core g of 8 owns q heads
4g..4g+3 and kv head g (columns of the qkv projection), plus the matching
512 input rows of w_out. Each core computes a partial y [2048,4096] (f16);
the host sums the 8 partials in fp32.

Optimizations vs the original baseline (427us -> ~403us):
  - softmax denominator off the PE: exp tiles are accumulated on the DVE
    into an fp32 running sum (final add emits bf16); one ones-matmul per
    (head, q-chunk) replaces a ones-matmul per k-tile (~61K PE cycles).
  - unified scheduler: attention for q-chunk qc starts as soon as s-tile
    4qc+3 is postprocessed; stage-1 dt-blocks (later, out-projection
    units) are emitted as PE filler between the score and PV matmuls of
    each k-tile step, so exp latency and postproc chains hide under real
    PE work and there are no stage-boundary bubbles. Remaining
    out-projection work forms a PE-bound tail.
  - q/k PE transposes are queued and emitted spaced-out so their
    PSUM-bank copies (alternating DVE/ACT) never block the PE.
  - PSUM rings: A(qkv acc + PV)x4, B(scores/den/trans)x2, C(kv acc /
    out-proj)x2; tail out-proj units alternate B/C for a 4-bank rotation.
  - partial outputs stored/DMAd as fp16 (host sums partials in fp32).
  - startup: st0/st1 matmuls emitted interleaved by dt-block so the PE
    tracks the weight-chunk DMA arrivals; critical first chunks issued
    across both DGE queues (sync + scalar).
  - gpsimd (Pool) takes SBUF-only rope work (it cannot access PSUM).
"""

import os
import sys
import types

import numpy as np

H = 32
G = 8
HD = 128
S = 2048
D = 4096
HG = H // G  # q heads per kv head = 4
EPS = 1e-5
THETA = 1e6
N_CORES = 8
ST = S // 128  # 16 s-tiles
DT = D // 128  # 32 d-tiles
QC = 4  # q chunks of 512


def _ensure_ntff_hook():
    """Register the axon NTFF profile hook if the image's antenv lacks it,
    so run_bass_kernel_spmd(trace=True) can return exec_time_ns."""
    try:
        from antenv.axon_hooks import get_axon_ntff_profile_hook  # noqa: F401
        return
    except ImportError:
        pass
    try:
        import antenv
        mod = types.ModuleType("antenv.axon_hooks")
        _h = [None]
        mod.set_axon_ntff_profile_hook = lambda h: _h.__setitem__(0, h)
        mod.get_axon_ntff_profile_hook = lambda: _h[0]
        sys.modules["antenv.axon_hooks"] = mod
        antenv.axon_hooks = mod
        from trn_agent_boot.trn_boot import _ntff_profile_via_ctypes
        so = "/opt/axon/libaxon_pjrt.so"
        if os.path.exists(so):
            mod.set_axon_ntff_profile_hook(_ntff_profile_via_ctypes(so))
    except Exception:
        pass


def _build_nc():
    import concourse.bass as bass  # noqa: F401
    import concourse.tile as tile
    from concourse import bacc, mybir

    bf16 = mybir.dt.bfloat16
    f16 = mybir.dt.float16
    f32 = mybir.dt.float32
    AF = mybir.ActivationFunctionType

    nc = bacc.Bacc("TRN2", target_bir_lowering=False, debug=False,
                   num_devices=N_CORES)

    # ---- DRAM I/O ----
    xt_d = nc.dram_tensor("xt", [ST, 128, DT, 128], bf16, kind="ExternalInput").ap()
    wqkv_d = nc.dram_tensor("wqkvT", [D, 768], bf16, kind="ExternalInput").ap()
    wo_d = nc.dram_tensor("woT", [512, D], bf16, kind="ExternalInput").ap()
    ccd_d = nc.dram_tensor("ccd", [S, 128], f16, kind="ExternalInput").ap()
    ssd_d = nc.dram_tensor("ssd", [S, 128], f16, kind="ExternalInput").ap()
    mask_d = nc.dram_tensor("dmask", [128, 128], bf16, kind="ExternalInput").ap()
    ident_d = nc.dram_tensor("ident", [128, 128], bf16, kind="ExternalInput").ap()
    out_d = nc.dram_tensor("out", [S, D], f16, kind="ExternalOutput").ap()

    from contextlib import ExitStack
    with tile.TileContext(nc) as tc, ExitStack() as ctx:
        const = ctx.enter_context(tc.tile_pool(name="const", bufs=1))
        persist = ctx.enter_context(tc.tile_pool(name="persist", bufs=1))
        xpool = ctx.enter_context(tc.tile_pool(name="xpool", bufs=3))
        scratch = ctx.enter_context(tc.tile_pool(name="scratch", bufs=2))
        small = ctx.enter_context(tc.tile_pool(name="small", bufs=2))
        epool = ctx.enter_context(tc.tile_pool(name="epool", bufs=7))
        spool = ctx.enter_context(tc.tile_pool(name="spool", bufs=4))
        opool = ctx.enter_context(tc.tile_pool(name="opool", bufs=2))
        psum = ctx.enter_context(tc.tile_pool(name="psum", bufs=3, space="PSUM"))

        # ---- critical path to first matmuls: wq chunks on the sync DGE
        # queue, x tiles + rope tables on the scalar DGE queue (parallel) ----
        wq_sb = persist.tile([128, DT, 768], bf16, tag="bigw")
        wq_r = wqkv_d.rearrange("(t p) e -> p t e", p=128)
        ccd_r = ccd_d.rearrange("(t p) h -> p t h", p=128)
        ssd_r = ssd_d.rearrange("(t p) h -> p t h", p=128)
        ccd_sb = const.tile([128, ST, 128], f16)
        ssd_sb = const.tile([128, ST, 128], f16)
        nc.sync.dma_start(out=wq_sb[:, 0:1, :], in_=wq_r[:, 0:1, :])
        xs_pre = []
        xs_p0 = xpool.tile([128, DT, 128], bf16, name="xs")
        nc.scalar.dma_start(out=xs_p0[:, 0:4, :], in_=xt_d[0, :, 0:4, :])
        xs_pre.append(xs_p0)
        nc.sync.dma_start(out=wq_sb[:, 1:2, :], in_=wq_r[:, 1:2, :])
        xs_p1 = xpool.tile([128, DT, 128], bf16, name="xs")
        nc.scalar.dma_start(out=xs_p1[:, 0:4, :], in_=xt_d[1, :, 0:4, :])
        xs_pre.append(xs_p1)
        nc.sync.dma_start(out=wq_sb[:, 2:4, :], in_=wq_r[:, 2:4, :])
        nc.scalar.dma_start(out=xs_p0[:, 4:16, :], in_=xt_d[0, :, 4:16, :])
        nc.scalar.dma_start(out=xs_p1[:, 4:16, :], in_=xt_d[1, :, 4:16, :])
        nc.sync.dma_start(out=wq_sb[:, 4:8, :], in_=wq_r[:, 4:8, :])
        nc.scalar.dma_start(out=xs_p0[:, 16:32, :], in_=xt_d[0, :, 16:32, :])
        nc.scalar.dma_start(out=xs_p1[:, 16:32, :], in_=xt_d[1, :, 16:32, :])
        # remaining weight chunks in fine 4-dt pieces so the PE's dt
        # progression tracks arrival; alternate queues for even drain
        for dtc in range(8, DT, 4):
            eng = nc.sync if (dtc // 4) % 2 == 0 else nc.scalar
            eng.dma_start(out=wq_sb[:, dtc:dtc + 4, :],
                          in_=wq_r[:, dtc:dtc + 4, :])
        # rope tables: first three s-tiles' worth first, rest behind
        nc.scalar.dma_start(out=ccd_sb[:, 0:3, :], in_=ccd_r[:, 0:3, :])
        nc.scalar.dma_start(out=ssd_sb[:, 0:3, :], in_=ssd_r[:, 0:3, :])
        xs_p2 = xpool.tile([128, DT, 128], bf16, name="xs")
        nc.scalar.dma_start(out=xs_p2[:, 0:16, :], in_=xt_d[2, :, 0:16, :])
        nc.scalar.dma_start(out=xs_p2[:, 16:32, :], in_=xt_d[2, :, 16:32, :])
        xs_pre.append(xs_p2)
        nc.scalar.dma_start(out=ccd_sb[:, 3:ST, :], in_=ccd_r[:, 3:ST, :])
        nc.scalar.dma_start(out=ssd_sb[:, 3:ST, :], in_=ssd_r[:, 3:ST, :])

        # ---- remaining constants ----
        mask_sb = const.tile([128, 128], bf16)
        nc.sync.dma_start(out=mask_sb, in_=mask_d)
        ident_sb = const.tile([128, 128], bf16)
        nc.sync.dma_start(out=ident_sb, in_=ident_d)
        onesm_sb = const.tile([128, 128], bf16)
        nc.vector.memset(onesm_sb, 1.0)
        bias_q = const.tile([128, 1], f32)
        nc.vector.memset(bias_q, float(HD * EPS))
        bias_k = const.tile([128, 1], f32)
        nc.vector.memset(bias_k, float(EPS))

        qT_sb = persist.tile([128, HG, S], bf16)   # [hd, head, s]
        kT_sb = persist.tile([128, S], bf16)       # [hd, s]
        v_sb = persist.tile([128, ST, 128], bf16)  # [s_local, s_tile, hd]
        oT_sb = persist.tile([128, HG, S], bf16)   # attn outT [hd, head, s]

        # ================= stage 1: qkv projection + postproc ==============
        def stage1_alloc(st):
            if st < 3:
                xs = xs_pre[st]
            else:
                xs = xpool.tile([128, DT, 128], bf16, name="xs")
                nc.sync.dma_start(out=xs, in_=xt_d[st])
            q_ps = psum.tile([128, 512], f32, tag="pa", bufs=4, name=f"q_ps_{st}")
            kv_ps = psum.tile([128, 512], f32, tag="pc", bufs=2, name=f"kv_ps_{st}")
            return xs, q_ps, kv_ps

        def stage1_mm_block(tile_state, dt0, dt1, parts="qkv"):
            xs, q_ps, kv_ps = tile_state
            for dt_i in range(dt0, dt1):
                if "q" in parts:
                    nc.tensor.matmul(q_ps, xs[:, dt_i, :], wq_sb[:, dt_i, 0:512],
                                     start=(dt_i == 0), stop=(dt_i == DT - 1))
                if "kv" in parts:
                    nc.tensor.matmul(kv_ps[:, 0:256], xs[:, dt_i, :],
                                     wq_sb[:, dt_i, 512:768],
                                     start=(dt_i == 0), stop=(dt_i == DT - 1))

        def stage1_matmuls(st):
            ts = stage1_alloc(st)
            stage1_mm_block(ts, 0, DT)
            return ts[1], ts[2]

        def stage1_postproc(st, q_ps, kv_ps):
            # v: straight cast copy to [s, hd]
            nc.vector.tensor_copy(out=v_sb[:, st, :], in_=kv_ps[:, 128:256])

            # sum of squares per head (ACT Square with free-dim accumulate)
            ssq = small.tile([128, 5], f32)
            sqs = scratch.tile([128, 512], f32)
            for hh in range(HG):
                nc.scalar.activation(out=sqs[:, hh * 128:(hh + 1) * 128],
                                     in_=q_ps[:, hh * 128:(hh + 1) * 128],
                                     func=AF.Square,
                                     accum_out=ssq[:, hh:hh + 1])
            sqk = small.tile([128, 128], f32)
            nc.scalar.activation(out=sqk, in_=kv_ps[:, 0:128], func=AF.Square,
                                 accum_out=ssq[:, 4:5])
            # rstd: q gets the 1/sqrt(HD) score scale folded in
            rstd = small.tile([128, 5], f32)
            nc.scalar.activation(out=rstd[:, 0:4], in_=ssq[:, 0:4],
                                 func=AF.Sqrt, bias=bias_q, scale=1.0)
            nc.scalar.activation(out=rstd[:, 4:5], in_=ssq[:, 4:5],
                                 func=AF.Sqrt, bias=bias_k, scale=1.0 / HD)
            nc.vector.reciprocal(out=rstd, in_=rstd)

            # rope q (4 heads batched; tables broadcast over head dim)
            q4 = q_ps.rearrange("p (h r two) -> p h r two", h=HG, two=2)
            rot_q = scratch.tile([128, HG, 64, 2], f32)
            nc.vector.tensor_copy(out=rot_q, in_=q4[:, :, :, ::-1])
            cc_b = ccd_sb[:, st, :].unsqueeze(1).broadcast_to((128, HG, 128))
            ss_b = ssd_sb[:, st, :].unsqueeze(1).broadcast_to((128, HG, 128))
            qcc = scratch.tile([128, HG, 128], f32)
            nc.vector.tensor_mul(qcc, q_ps.rearrange("p (h e) -> p h e", h=HG), cc_b)
            qss = scratch.tile([128, HG, 128], f32)
            nc.gpsimd.tensor_mul(qss, rot_q.rearrange("p h r two -> p h (r two)"), ss_b)
            qrope = scratch.tile([128, HG, 128], f32)
            nc.gpsimd.tensor_add(qrope, qcc, qss)
            qfin = scratch.tile([128, HG, 128], bf16, bufs=3)
            for hh in range(HG):
                nc.vector.tensor_scalar_mul(qfin[:, hh, :], qrope[:, hh, :],
                                            rstd[:, hh:hh + 1])

            # rope k
            k1 = kv_ps[:, 0:128].rearrange("p (r two) -> p r two", two=2)
            rot_k = small.tile([128, 64, 2], f32)
            nc.vector.tensor_copy(out=rot_k, in_=k1[:, :, ::-1])
            kcc = small.tile([128, 128], f32)
            nc.vector.tensor_mul(kcc, kv_ps[:, 0:128], ccd_sb[:, st, :])
            kss = small.tile([128, 128], f32)
            nc.gpsimd.tensor_mul(kss, rot_k.rearrange("p r two -> p (r two)"),
                                 ssd_sb[:, st, :])
            krope = small.tile([128, 128], f32)
            nc.gpsimd.tensor_add(krope, kcc, kss)
            kfin = small.tile([128, 128], bf16, bufs=3)
            nc.vector.tensor_scalar_mul(kfin, krope, rstd[:, 4:5])
            return qfin, kfin

        # q/k transposes (PE) are queued and emitted SPACED between other
        # PE work so their PSUM-bank copies never block the PE
        trans_q = []   # (st, 'q'|'k', tile, head)
        tcnt = [0]

        def stage1_transposes(st, qfin, kfin):
            for hh in range(HG):
                trans_q.append((st, "q", qfin, hh))
            trans_q.append((st, "k", kfin, None))

        def emit_trans_item():
            st, kind, buf, hh = trans_q.pop(0)
            t_ps = psum.tile([128, 128], bf16, tag="pb", bufs=2, name="t_ps")
            if kind == "q":
                nc.tensor.transpose(t_ps, buf[:, hh, :], ident_sb)
                dst = qT_sb[:, hh, st * 128:(st + 1) * 128]
            else:
                nc.tensor.transpose(t_ps, buf, ident_sb)
                dst = kT_sb[:, st * 128:(st + 1) * 128]
            tcnt[0] += 1
            if tcnt[0] % 2 == 0:
                nc.vector.tensor_copy(out=dst, in_=t_ps)
            else:
                nc.scalar.copy(out=dst, in_=t_ps)

        # ====== unified scheduler: stage-1 / attention / out-proj ======
        # Stage-1 tiles are emitted in 4-dt blocks. Once tile 4qc+3 is
        # postprocessed, q-chunk qc's two attention head-pairs unlock and
        # run with stage-1 blocks (later out-proj units) as PE filler
        # between the score and PV matmuls of each k-tile step. All
        # remaining out-proj work forms a PE-bound tail.

        # st0/st1 matmuls interleaved up front so the PE tracks the
        # weight-chunk DMA arrivals instead of stalling on st0's tail
        ts0 = stage1_alloc(0)
        ts1 = stage1_alloc(1)
        for dtb in range(0, DT, 4):
            stage1_mm_block(ts0, dtb, dtb + 4)
            stage1_mm_block(ts1, dtb, dtb + 4)
        qf0, kf0 = stage1_postproc(0, ts0[1], ts0[2])
        qf1, kf1 = stage1_postproc(1, ts1[1], ts1[2])
        stage1_transposes(0, qf0, kf0)
        stage1_transposes(1, qf1, kf1)

        pend_attn = []        # (qc, hp) pairs ready to emit
        attn_done_qcs = []    # q-chunks whose oT is fully written
        s1 = {"st": 2, "blk": 0, "ts": None, "s1_done": False}

        # out-projection work queue: units of (st, half, ec). Each unit is
        # 4 accumulating matmuls (one per head) into one PSUM bank, then a
        # copy (alternating DVE/ACT) into the staging buffer; one DMA per
        # (st, half).
        wout_q = []      # pending units
        out_stage = {}   # (st, half) -> staging tile
        done_units = {}  # (st, half) -> count of copied units
        stream2 = {}     # (st, half) -> ship output in 1024-col halves
        unit_no = [0]    # emitted-unit counter (copy-engine parity)

        def emit_wout_unit(tail=False):
            st, half, i = wout_q.pop(0)
            ec = half * 4 + i
            key = (st, half)
            if key not in out_stage:
                out_stage[key] = opool.tile([128, 2048], f16, name="ost", tag="ost")
                done_units[key] = 0
                stream2[key] = tail and len(wout_q) <= 8
            # in the tail (attention finished) alternate with the pb ring so
            # the unit pipeline rotates over four banks instead of two
            tag = "pb" if tail and unit_no[0] % 2 == 0 else "pc"
            o_ps = psum.tile([128, 512], f32, tag=tag, bufs=2,
                             name=f"o_ps_{st}_{half}_{i}")
            for h in range(HG):
                nc.tensor.matmul(o_ps,
                                 oT_sb[:, h, st * 128:(st + 1) * 128],
                                 wo_sb[:, h, ec * 512:(ec + 1) * 512],
                                 start=(h == 0), stop=(h == HG - 1))
            ost = out_stage[key]
            unit_no[0] += 1
            if unit_no[0] % 2 == 0:
                nc.vector.tensor_copy(out=ost[:, i * 512:(i + 1) * 512], in_=o_ps)
            else:
                nc.scalar.copy(out=ost[:, i * 512:(i + 1) * 512], in_=o_ps)
            done_units[key] += 1
            if stream2[key]:
                # final groups: ship each 1024-column half as soon as its
                # two units have landed, so the drain tail is short
                if done_units[key] in (2, 4):
                    c0 = 0 if done_units[key] == 2 else 1024
                    nc.sync.dma_start(
                        out=out_d[st * 128:(st + 1) * 128,
                                  half * 2048 + c0:half * 2048 + c0 + 1024],
                        in_=ost[:, c0:c0 + 1024])
                    if done_units[key] == 4:
                        del out_stage[key]
            elif done_units[key] == 4:
                nc.sync.dma_start(
                    out=out_d[st * 128:(st + 1) * 128,
                              half * 2048:(half + 1) * 2048],
                    in_=ost)
                del out_stage[key]

        wo_sb = None

        def wo_dma():
            # stage-3 weights reuse wq_sb's SBUF slot (same tag); the WAR
            # dep on st15's matmuls delays this DMA, so split it by
            # e-column range to let the first out-proj units start early
            nonlocal wo_sb
            wo_sb = persist.tile([128, HG, D], bf16, tag="bigw")
            wo_r = wo_d.rearrange("(h p) e -> p h e", p=128)
            for ecc in range(0, 8, 2):
                nc.sync.dma_start(out=wo_sb[:, :, ecc * 512:(ecc + 2) * 512],
                                  in_=wo_r[:, :, ecc * 512:(ecc + 2) * 512])

        def push_wout(qc):
            for st in range(4 * qc, 4 * qc + 4):
                for half in range(2):
                    for i in range(4):
                        wout_q.append((st, half, i))

        def s1_emit_block():
            st = s1["st"]
            if st >= ST:
                return
            # only emit transposes whose postproc chain has had a full tile
            # of PE time to complete; one per block so the next score matmul
            # never inherits a bank still waiting on a transpose copy
            if trans_q and trans_q[0][0] <= st - 2:
                emit_trans_item()
            if s1["blk"] == 0:
                s1["ts"] = stage1_alloc(st)
            b = s1["blk"]
            stage1_mm_block(s1["ts"], 4 * b, 4 * b + 4)
            if st == ST - 1 and 4 * b + 4 == DT:
                wo_dma()
            s1["blk"] += 1
            if s1["blk"] == DT // 4:
                s1["blk"] = 0
                s1["st"] += 1
                ts = s1["ts"]
                qfin, kfin = stage1_postproc(st, ts[1], ts[2])
                stage1_transposes(st, qfin, kfin)
                if st % 4 == 3:
                    qc = (st - 3) // 4
                    pend_attn.append((qc, 0))
                    pend_attn.append((qc, 1))
                if st == ST - 1:
                    s1["s1_done"] = True
                    for qc in attn_done_qcs:
                        push_wout(qc)

        wo_hold = [16]  # filler slots to skip while wo weights are in flight

        def emit_filler(n_s1, n_wout):
            while n_s1 > 0 and s1["st"] < ST:
                s1_emit_block()
                n_s1 -= 1
            if s1["st"] >= ST:
                for _ in range(min(2, len(trans_q))):
                    emit_trans_item()
                if wo_hold[0] > 0:
                    wo_hold[0] -= 1
                    return
            while n_wout > 0 and wout_q:
                emit_wout_unit()
                n_wout -= 1

        def run_attn_hp(qc, hp):
                while trans_q and trans_q[0][0] <= 4 * qc + 3:
                    emit_trans_item()
                if hp == 0 and s1["st"] < ST:
                    # a couple of stage-1 blocks of PE time for the fresh qT
                    # copies to land before the first score reads them
                    emit_filler(2, 0)
                hh0 = 2 * hp
                pv0 = psum.tile([128, 512], f32, tag="pa", bufs=4, name=f"pv0_{qc}_{hp}")
                pv1 = psum.tile([128, 512], f32, tag="pa", bufs=4, name=f"pv1_{qc}_{hp}")
                pvs = [pv0, pv1]
                exsum = [spool.tile([128, 512], f32, tag="exs", bufs=2,
                                    name=f"exsum_{qc}_{hp}_{hi}") for hi in range(2)]
                exsum_b = [spool.tile([128, 512], bf16, tag="exsb", bufs=2,
                                      name=f"exsumb_{qc}_{hp}_{hi}") for hi in range(2)]
                n_kt = 4 * qc + 4
                for kt in range(n_kt):
                    j = kt - 4 * qc
                    off = 0 if j < 0 else 128 * j
                    exs = []
                    for hi in range(2):
                        h = hh0 + hi
                        sc_ps = psum.tile([128, 512], f32, tag="pb", bufs=2,
                                          name=f"sc_{qc}_{hp}_{kt}_{hi}")
                        nc.tensor.matmul(
                            sc_ps[:, off:512],
                            kT_sb[:, kt * 128:(kt + 1) * 128],
                            qT_sb[:, h, qc * 512 + off:(qc + 1) * 512],
                            start=True, stop=True)
                        ex = epool.tile([128, 512], bf16, name=f"ex_{hi}")
                        nc.scalar.activation(out=ex[:, off:512],
                                             in_=sc_ps[:, off:512], func=AF.Exp)
                        if j >= 0:
                            nc.vector.tensor_mul(ex[:, off:off + 128],
                                                 ex[:, off:off + 128], mask_sb)
                        exs.append(ex)
                    # denominator partial sums on the DVE (keeps them off
                    # the PE); the diagonal-tile prefix copy goes to the Pool
                    for hi in range(2):
                        if kt == 0:
                            nc.vector.tensor_copy(out=exsum[hi][:, off:512],
                                                  in_=exs[hi][:, off:512])
                        elif kt < n_kt - 1:
                            nc.vector.tensor_add(exsum[hi][:, off:512],
                                                 exsum[hi][:, off:512],
                                                 exs[hi][:, off:512])
                        else:
                            if off > 0:
                                nc.vector.tensor_copy(out=exsum_b[hi][:, 0:off],
                                                      in_=exsum[hi][:, 0:off])
                            nc.vector.tensor_add(exsum_b[hi][:, off:512],
                                                 exsum[hi][:, off:512],
                                                 exs[hi][:, off:512])
                    # PE filler between scores and PV hides exp latency:
                    # stage-1 blocks while they last, out-proj units after
                    emit_filler(2, 1)
                    for hi in range(2):
                        nc.tensor.matmul(pvs[hi][:, off:512], v_sb[:, kt, :],
                                         exs[hi][:, off:512],
                                         start=(kt == 0), stop=(kt == n_kt - 1))
                # PE filler so the denominators' exp/add chain can finish
                emit_filler(1, 1)
                for hi in range(2):
                    h = hh0 + hi
                    den_ps = psum.tile([128, 512], f32, tag="pb", bufs=2,
                                       name=f"den_{qc}_{hp}_{hi}")
                    nc.tensor.matmul(den_ps, onesm_sb, exsum_b[hi],
                                     start=True, stop=True)
                    rden = scratch.tile([128, 512], f32, tag="rden")
                    nc.vector.reciprocal_approx_fast(out=rden, in_=den_ps)
                    nc.vector.tensor_mul(oT_sb[:, h, qc * 512:(qc + 1) * 512],
                                         pvs[hi], rden)


        while pend_attn or not s1["s1_done"] or wout_q:
            if pend_attn:
                qc, hp = pend_attn.pop(0)
                run_attn_hp(qc, hp)
                if hp == 1:
                    attn_done_qcs.append(qc)
                    if s1["s1_done"]:
                        push_wout(qc)
            elif not s1["s1_done"]:
                s1_emit_block()
            else:
                emit_wout_unit(tail=True)

    nc.compile()
    return nc


def _host_prep(x, w_qkv, w_out, q_ln_w, k_ln_w):
    """Build per-core input maps (host-side shard + transform)."""
    import ml_dtypes
    bf16 = ml_dtypes.bfloat16

    x2 = np.asarray(x, np.float32).reshape(S, D)
    # x tiles [st, d_local, d_tile, s_local] so each s-tile DMA is contiguous
    xt = np.ascontiguousarray(
        x2.reshape(ST, 128, DT, 128).transpose(0, 3, 2, 1)).astype(bf16)

    # rope tables (duplicated cos / sign-baked sin, interleaved layout)
    freqs = 1.0 / (THETA ** (np.arange(0, HD, 2, dtype=np.float64) / HD))
    ang = np.arange(S, dtype=np.float64)[:, None] * freqs[None, :]
    cos = np.cos(ang).astype(np.float32)
    sin = np.sin(ang).astype(np.float32)
    ccd = np.repeat(cos, 2, axis=1).astype(np.float16)    # [S, 128]
    ssd = np.stack([-sin, sin], axis=-1).reshape(S, HD).astype(np.float16)

    kq = np.arange(128)
    dmask = (kq[:, None] <= kq[None, :]).astype(bf16)     # [k, q]
    ident = np.eye(128, dtype=bf16)

    wq = np.asarray(w_qkv, np.float32)
    wo = np.asarray(w_out, np.float32)
    qw = np.asarray(q_ln_w, np.float32)
    kw = np.asarray(k_ln_w, np.float32)

    in_maps = []
    for g in range(N_CORES):
        wq_g = wq[512 * g:512 * (g + 1), :].reshape(HG, HD, D) * qw[None, :, None]
        wk_g = wq[D + 128 * g:D + 128 * (g + 1), :] * kw[:, None]
        wv_g = wq[D + G * HD + 128 * g:D + G * HD + 128 * (g + 1), :]
        wqkv_g = np.concatenate([wq_g.reshape(512, D), wk_g, wv_g], axis=0)
        wqkvT_g = np.ascontiguousarray(wqkv_g.T).astype(bf16)     # [D, 768]
        woT_g = np.ascontiguousarray(wo[:, 512 * g:512 * (g + 1)].T).astype(bf16)
        in_maps.append({
            "xt": xt,
            "wqkvT": wqkvT_g,
            "woT": woT_g,
            "ccd": ccd,
            "ssd": ssd,
            "dmask": dmask,
            "ident": ident,
        })
    return in_maps


_CACHE = {}


def _get_compiled():
    if "nc" not in _CACHE:
        _ensure_ntff_hook()
        _CACHE["nc"] = _build_nc()
    return _CACHE["nc"]


def run_sharded(x, w_qkv, w_out, q_ln_w, k_ln_w, trace=False):
    from concourse.bass_utils import run_bass_kernel_spmd
    nc = _get_compiled()
    in_maps = _host_prep(x, w_qkv, w_out, q_ln_w, k_ln_w)
    res = run_bass_kernel_spmd(nc, in_maps, core_ids=list(range(N_CORES)),
                               trace=trace)
    acc = np.zeros((S, D), np.float32)
    for i in range(N_CORES):
        acc += np.asarray(res.results[i]["out"], np.float32)
    return acc.reshape(1, S, D), res


def kernel(x, w_qkv, w_out, q_ln_w, k_ln_w):
    out, _ = run_sharded(x, w_qkv, w_out, q_ln_w, k_ln_w, trace=False)
    return out


# revision 33
# speedup vs baseline: 1.0090x; 1.0090x over previous
"""Trainium2 Bass kernel for GQA multi-head attention block (nn_MHA_68831145886222).

Computation (reference):
  qkv = x @ w_qkv.T ; split q[32 heads],k[8],v[8] (HD=128)
  q,k = rmsnorm(head_dim) -> rope(interleaved, theta=1e6)
  out = causal GQA attention (4 q heads per kv head)
  y   = (attn out) @ w_out.T

Sharding: tensor-parallel by kv-head group. Core g of 8 owns q heads
4g..4g+3 and kv head g (columns of the qkv projection), plus the matching
512 input rows of w_out. Each core computes a partial y [2048,4096] (f16);
the host sums the 8 partials in fp32.

Optimizations vs the original baseline (427us -> ~403us):
  - softmax denominator off the PE: exp tiles are accumulated on the DVE
    into an fp32 running sum (final add emits bf16); one ones-matmul per
    (head, q-chunk) replaces a ones-matmul per k-tile (~61K PE cycles).
  - unified scheduler: attention for q-chunk qc starts as soon as s-tile
    4qc+3 is postprocessed; stage-1 dt-blocks (later, out-projection
    units) are emitted as PE filler between the score and PV matmuls of
    each k-tile step, so exp latency and postproc chains hide under real
    PE work and there are no stage-boundary bubbles. Remaining
    out-projection work forms a PE-bound tail.
  - q/k PE transposes are queued and emitted spaced-out so their
    PSUM-bank copies (alternating DVE/ACT) never block the PE.
  - PSUM rings: A(qkv acc + PV)x4, B(scores/den/trans)x2, C(kv acc /
    out-proj)x2; tail out-proj units alternate B/C for a 4-bank rotation.
  - partial outputs stored/DMAd as fp16 (host sums partials in fp32).
  - startup: st0/st1 matmuls emitted interleaved by dt-block so the PE
    tracks the weight-chunk DMA arrivals; critical first chunks issued
    across both DGE queues (sync + scalar).
  - gpsimd (Pool) takes SBUF-only rope work (it cannot access PSUM).
"""

import os
import sys
import types

import numpy as np

H = 32
G = 8
HD = 128
S = 2048
D = 4096
HG = H // G  # q heads per kv head = 4
EPS = 1e-5
THETA = 1e6
N_CORES = 8
ST = S // 128  # 16 s-tiles
DT = D // 128  # 32 d-tiles
QC = 4  # q chunks of 512


def _ensure_ntff_hook():
    """Register the axon NTFF profile hook if the image's antenv lacks it,
    so run_bass_kernel_spmd(trace=True) can return exec_time_ns."""
    try:
        from antenv.axon_hooks import get_axon_ntff_profile_hook  # noqa: F401
        return
    except ImportError:
        pass
    try:
        import antenv
        mod = types.ModuleType("antenv.axon_hooks")
        _h = [None]
        mod.set_axon_ntff_profile_hook = lambda h: _h.__setitem__(0, h)
        mod.get_axon_ntff_profile_hook = lambda: _h[0]
        sys.modules["antenv.axon_hooks"] = mod
        antenv.axon_hooks = mod
        from trn_agent_boot.trn_boot import _ntff_profile_via_ctypes
        so = "/opt/axon/libaxon_pjrt.so"
        if os.path.exists(so):
            mod.set_axon_ntff_profile_hook(_ntff_profile_via_ctypes(so))
    except Exception:
        pass


def _build_nc():
    import concourse.bass as bass  # noqa: F401
    import concourse.tile as tile
    from concourse import bacc, mybir

    bf16 = mybir.dt.bfloat16
    f16 = mybir.dt.float16
    f32 = mybir.dt.float32
    AF = mybir.ActivationFunctionType

    nc = bacc.Bacc("TRN2", target_bir_lowering=False, debug=False,
                   num_devices=N_CORES)

    # ---- DRAM I/O ----
    xt_d = nc.dram_tensor("xt", [ST, 128, DT, 128], bf16, kind="ExternalInput").ap()
    wqkv_d = nc.dram_tensor("wqkvT", [D, 768], bf16, kind="ExternalInput").ap()
    wo_d = nc.dram_tensor("woT", [512, D], bf16, kind="ExternalInput").ap()
    ccd_d = nc.dram_tensor("ccd", [S, 128], f16, kind="ExternalInput").ap()
    ssd_d = nc.dram_tensor("ssd", [S, 128], f16, kind="ExternalInput").ap()
    mask_d = nc.dram_tensor("dmask", [128, 128], bf16, kind="ExternalInput").ap()
    ident_d = nc.dram_tensor("ident", [128, 128], bf16, kind="ExternalInput").ap()
    out_d = nc.dram_tensor("out", [S, D], f16, kind="ExternalOutput").ap()

    from contextlib import ExitStack
    with tile.TileContext(nc) as tc, ExitStack() as ctx:
        const = ctx.enter_context(tc.tile_pool(name="const", bufs=1))
        persist = ctx.enter_context(tc.tile_pool(name="persist", bufs=1))
        xpool = ctx.enter_context(tc.tile_pool(name="xpool", bufs=3))
        scratch = ctx.enter_context(tc.tile_pool(name="scratch", bufs=2))
        small = ctx.enter_context(tc.tile_pool(name="small", bufs=2))
        epool = ctx.enter_context(tc.tile_pool(name="epool", bufs=7))
        spool = ctx.enter_context(tc.tile_pool(name="spool", bufs=4))
        opool = ctx.enter_context(tc.tile_pool(name="opool", bufs=2))
        psum = ctx.enter_context(tc.tile_pool(name="psum", bufs=3, space="PSUM"))

        # ---- critical path to first matmuls: wq chunks on the sync DGE
        # queue, x tiles + rope tables on the scalar DGE queue (parallel) ----
        wq_sb = persist.tile([128, DT, 768], bf16, tag="bigw")
        wq_r = wqkv_d.rearrange("(t p) e -> p t e", p=128)
        ccd_r = ccd_d.rearrange("(t p) h -> p t h", p=128)
        ssd_r = ssd_d.rearrange("(t p) h -> p t h", p=128)
        ccd_sb = const.tile([128, ST, 128], f16)
        ssd_sb = const.tile([128, ST, 128], f16)
        nc.sync.dma_start(out=wq_sb[:, 0:1, :], in_=wq_r[:, 0:1, :])
        xs_pre = []
        xs_p0 = xpool.tile([128, DT, 128], bf16, name="xs")
        nc.scalar.dma_start(out=xs_p0[:, 0:4, :], in_=xt_d[0, :, 0:4, :])
        xs_pre.append(xs_p0)
        nc.sync.dma_start(out=wq_sb[:, 1:2, :], in_=wq_r[:, 1:2, :])
        xs_p1 = xpool.tile([128, DT, 128], bf16, name="xs")
        nc.scalar.dma_start(out=xs_p1[:, 0:4, :], in_=xt_d[1, :, 0:4, :])
        xs_pre.append(xs_p1)
        nc.sync.dma_start(out=wq_sb[:, 2:4, :], in_=wq_r[:, 2:4, :])
        nc.scalar.dma_start(out=xs_p0[:, 4:16, :], in_=xt_d[0, :, 4:16, :])
        nc.scalar.dma_start(out=xs_p1[:, 4:16, :], in_=xt_d[1, :, 4:16, :])
        nc.sync.dma_start(out=wq_sb[:, 4:8, :], in_=wq_r[:, 4:8, :])
        nc.scalar.dma_start(out=xs_p0[:, 16:32, :], in_=xt_d[0, :, 16:32, :])
        nc.scalar.dma_start(out=xs_p1[:, 16:32, :], in_=xt_d[1, :, 16:32, :])
        # remaining weight chunks in fine 4-dt pieces so the PE's dt
        # progression tracks arrival; alternate queues for even drain
        for dtc in range(8, DT, 4):
            eng = nc.sync if (dtc // 4) % 2 == 0 else nc.scalar
            eng.dma_start(out=wq_sb[:, dtc:dtc + 4, :],
                          in_=wq_r[:, dtc:dtc + 4, :])
        # rope tables: first three s-tiles' worth first, rest behind
        nc.scalar.dma_start(out=ccd_sb[:, 0:3, :], in_=ccd_r[:, 0:3, :])
        nc.scalar.dma_start(out=ssd_sb[:, 0:3, :], in_=ssd_r[:, 0:3, :])
        xs_p2 = xpool.tile([128, DT, 128], bf16, name="xs")
        nc.scalar.dma_start(out=xs_p2[:, 0:16, :], in_=xt_d[2, :, 0:16, :])
        nc.scalar.dma_start(out=xs_p2[:, 16:32, :], in_=xt_d[2, :, 16:32, :])
        xs_pre.append(xs_p2)
        nc.scalar.dma_start(out=ccd_sb[:, 3:ST, :], in_=ccd_r[:, 3:ST, :])
        nc.scalar.dma_start(out=ssd_sb[:, 3:ST, :], in_=ssd_r[:, 3:ST, :])

        # ---- remaining constants ----
        mask_sb = const.tile([128, 128], bf16)
        nc.sync.dma_start(out=mask_sb, in_=mask_d)
        ident_sb = const.tile([128, 128], bf16)
        nc.sync.dma_start(out=ident_sb, in_=ident_d)
        onesm_sb = const.tile([128, 128], bf16)
        nc.vector.memset(onesm_sb, 1.0)
        bias_q = const.tile([128, 1], f32)
        nc.vector.memset(bias_q, float(HD * EPS))
        bias_k = const.tile([128, 1], f32)
        nc.vector.memset(bias_k, float(EPS))

        qT_sb = persist.tile([128, HG, S], bf16)   # [hd, head, s]
        kT_sb = persist.tile([128, S], bf16)       # [hd, s]
        v_sb = persist.tile([128, ST, 128], bf16)  # [s_local, s_tile, hd]
        oT_sb = persist.tile([128, HG, S], bf16)   # attn outT [hd, head, s]

        # ================= stage 1: qkv projection + postproc ==============
        def stage1_alloc(st):
            if st < 3:
                xs = xs_pre[st]
            else:
                xs = xpool.tile([128, DT, 128], bf16, name="xs")
                nc.sync.dma_start(out=xs, in_=xt_d[st])
            q_ps = psum.tile([128, 512], f32, tag="pa", bufs=4, name=f"q_ps_{st}")
            kv_ps = psum.tile([128, 512], f32, tag="pc", bufs=2, name=f"kv_ps_{st}")
            return xs, q_ps, kv_ps

        def stage1_mm_block(tile_state, dt0, dt1, parts="qkv"):
            xs, q_ps, kv_ps = tile_state
            for dt_i in range(dt0, dt1):
                if "q" in parts:
                    nc.tensor.matmul(q_ps, xs[:, dt_i, :], wq_sb[:, dt_i, 0:512],
                                     start=(dt_i == 0), stop=(dt_i == DT - 1))
                if "kv" in parts:
                    nc.tensor.matmul(kv_ps[:, 0:256], xs[:, dt_i, :],
                                     wq_sb[:, dt_i, 512:768],
                                     start=(dt_i == 0), stop=(dt_i == DT - 1))

        def stage1_matmuls(st):
            ts = stage1_alloc(st)
            stage1_mm_block(ts, 0, DT)
            return ts[1], ts[2]

        def stage1_postproc(st, q_ps, kv_ps):
            # v: straight cast copy to [s, hd]
            nc.vector.tensor_copy(out=v_sb[:, st, :], in_=kv_ps[:, 128:256])

            # sum of squares per head (ACT Square with free-dim accumulate)
            ssq = small.tile([128, 5], f32)
            sqs = scratch.tile([128, 512], f32)
            for hh in range(HG):
                nc.scalar.activation(out=sqs[:, hh * 128:(hh + 1) * 128],
                                     in_=q_ps[:, hh * 128:(hh + 1) * 128],
                                     func=AF.Square,
                                     accum_out=ssq[:, hh:hh + 1])
            sqk = small.tile([128, 128], f32)
            nc.scalar.activation(out=sqk, in_=kv_ps[:, 0:128], func=AF.Square,
                                 accum_out=ssq[:, 4:5])
            # rstd: q gets the 1/sqrt(HD) score scale folded in
            rstd = small.tile([128, 5], f32)
            nc.scalar.activation(out=rstd[:, 0:4], in_=ssq[:, 0:4],
                                 func=AF.Sqrt, bias=bias_q, scale=1.0)
            nc.scalar.activation(out=rstd[:, 4:5], in_=ssq[:, 4:5],
                                 func=AF.Sqrt, bias=bias_k, scale=1.0 / HD)
            nc.vector.reciprocal(out=rstd, in_=rstd)

            # rope q (4 heads batched; tables broadcast over head dim)
            q4 = q_ps.rearrange("p (h r two) -> p h r two", h=HG, two=2)
            rot_q = scratch.tile([128, HG, 64, 2], f32)
            nc.vector.tensor_copy(out=rot_q, in_=q4[:, :, :, ::-1])
            cc_b = ccd_sb[:, st, :].unsqueeze(1).broadcast_to((128, HG, 128))
            ss_b = ssd_sb[:, st, :].unsqueeze(1).broadcast_to((128, HG, 128))
            qcc = scratch.tile([128, HG, 128], f32)
            nc.vector.tensor_mul(qcc, q_ps.rearrange("p (h e) -> p h e", h=HG), cc_b)
            qss = scratch.tile([128, HG, 128], f32)
            nc.gpsimd.tensor_mul(qss, rot_q.rearrange("p h r two -> p h (r two)"), ss_b)
            qrope = scratch.tile([128, HG, 128], f32)
            nc.gpsimd.tensor_add(qrope, qcc, qss)
            qfin = scratch.tile([128, HG, 128], bf16, bufs=3)
            for hh in range(HG):
                nc.vector.tensor_scalar_mul(qfin[:, hh, :], qrope[:, hh, :],
                                            rstd[:, hh:hh + 1])

            # rope k
            k1 = kv_ps[:, 0:128].rearrange("p (r two) -> p r two", two=2)
            rot_k = small.tile([128, 64, 2], f32)
            nc.vector.tensor_copy(out=rot_k, in_=k1[:, :, ::-1])
            kcc = small.tile([128, 128], f32)
            nc.vector.tensor_mul(kcc, kv_ps[:, 0:128], ccd_sb[:, st, :])
            kss = small.tile([128, 128], f32)
            nc.gpsimd.tensor_mul(kss, rot_k.rearrange("p r two -> p (r two)"),
                                 ssd_sb[:, st, :])
            krope = small.tile([128, 128], f32)
            nc.gpsimd.tensor_add(krope, kcc, kss)
            kfin = small.tile([128, 128], bf16, bufs=3)
            nc.vector.tensor_scalar_mul(kfin, krope, rstd[:, 4:5])
            return qfin, kfin

        # q/k transposes (PE) are queued and emitted SPACED between other
        # PE work so their PSUM-bank copies never block the PE
        trans_q = []   # (st, 'q'|'k', tile, head)
        tcnt = [0]

        def stage1_transposes(st, qfin, kfin):
            for hh in range(HG):
                trans_q.append((st, "q", qfin, hh))
            trans_q.append((st, "k", kfin, None))

        def emit_trans_item():
            st, kind, buf, hh = trans_q.pop(0)
            t_ps = psum.tile([128, 128], bf16, tag="pb", bufs=2, name="t_ps")
            if kind == "q":
                nc.tensor.transpose(t_ps, buf[:, hh, :], ident_sb)
                dst = qT_sb[:, hh, st * 128:(st + 1) * 128]
            else:
                nc.tensor.transpose(t_ps, buf, ident_sb)
                dst = kT_sb[:, st * 128:(st + 1) * 128]
            tcnt[0] += 1
            if tcnt[0] % 2 == 0:
                nc.vector.tensor_copy(out=dst, in_=t_ps)
            else:
                nc.scalar.copy(out=dst, in_=t_ps)

        # ====== unified scheduler: stage-1 / attention / out-proj ======
        # Stage-1 tiles are emitted in 4-dt blocks. Once tile 4qc+3 is
        # postprocessed, q-chunk qc's two attention head-pairs unlock and
        # run with stage-1 blocks (later out-proj units) as PE filler
        # between the score and PV matmuls of each k-tile step. All
        # remaining out-proj work forms a PE-bound tail.

        # st0/st1 matmuls interleaved up front so the PE tracks the
        # weight-chunk DMA arrivals instead of stalling on st0's tail
        ts0 = stage1_alloc(0)
        ts1 = stage1_alloc(1)
        for dtb in range(0, DT, 4):
            stage1_mm_block(ts0, dtb, dtb + 4)
            stage1_mm_block(ts1, dtb, dtb + 4)
        qf0, kf0 = stage1_postproc(0, ts0[1], ts0[2])
        qf1, kf1 = stage1_postproc(1, ts1[1], ts1[2])
        stage1_transposes(0, qf0, kf0)
        stage1_transposes(1, qf1, kf1)

        pend_attn = []        # (qc, hp) pairs ready to emit
        attn_done_qcs = []    # q-chunks whose oT is fully written
        s1 = {"st": 2, "blk": 0, "ts": None, "s1_done": False}

        # out-projection work queue: units of (st, half, ec). Each unit is
        # 4 accumulating matmuls (one per head) into one PSUM bank, then a
        # copy (alternating DVE/ACT) into the staging buffer; one DMA per
        # (st, half).
        wout_q = []      # pending units
        out_stage = {}   # (st, half) -> staging tile
        done_units = {}  # (st, half) -> count of copied units
        stream2 = {}     # (st, half) -> ship output in 1024-col halves
        unit_no = [0]    # emitted-unit counter (copy-engine parity)

        def emit_wout_unit(tail=False):
            st, half, i = wout_q.pop(0)
            ec = half * 4 + i
            key = (st, half)
            if key not in out_stage:
                out_stage[key] = opool.tile([128, 2048], f16, name="ost", tag="ost")
                done_units[key] = 0
                stream2[key] = tail and len(wout_q) <= 8
            # in the tail (attention finished) alternate with the pb ring so
            # the unit pipeline rotates over four banks instead of two
            tag = "pb" if tail and unit_no[0] % 2 == 0 else "pc"
            o_ps = psum.tile([128, 512], f32, tag=tag, bufs=2,
                             name=f"o_ps_{st}_{half}_{i}")
            for h in range(HG):
                nc.tensor.matmul(o_ps,
                                 oT_sb[:, h, st * 128:(st + 1) * 128],
                                 wo_sb[:, h, ec * 512:(ec + 1) * 512],
                                 start=(h == 0), stop=(h == HG - 1))
            ost = out_stage[key]
            unit_no[0] += 1
            if unit_no[0] % 2 == 0:
                nc.vector.tensor_copy(out=ost[:, i * 512:(i + 1) * 512], in_=o_ps)
            else:
                nc.scalar.copy(out=ost[:, i * 512:(i + 1) * 512], in_=o_ps)
            done_units[key] += 1
            if stream2[key]:
                # final groups: ship each 1024-column half as soon as its
                # two units have landed, so the drain tail is short
                if done_units[key] in (2, 4):
                    c0 = 0 if done_units[key] == 2 else 1024
                    nc.sync.dma_start(
                        out=out_d[st * 128:(st + 1) * 128,
                                  half * 2048 + c0:half * 2048 + c0 + 1024],
                        in_=ost[:, c0:c0 + 1024])
                    if done_units[key] == 4:
                        del out_stage[key]
            elif done_units[key] == 4:
                nc.sync.dma_start(
                    out=out_d[st * 128:(st + 1) * 128,
                              half * 2048:(half + 1) * 2048],
                    in_=ost)
                del out_stage[key]

        wo_sb = None

        def wo_dma():
            # stage-3 weights reuse wq_sb's SBUF slot (same tag); the WAR
            # dep on st15's matmuls delays this DMA, so split it by
            # e-column range to let the first out-proj units start early
            nonlocal wo_sb
            wo_sb = persist.tile([128, HG, D], bf16, tag="bigw")
            wo_r = wo_d.rearrange("(h p) e -> p h e", p=128)
            for ecc in range(0, 8, 2):
                nc.sync.dma_start(out=wo_sb[:, :, ecc * 512:(ecc + 2) * 512],
                                  in_=wo_r[:, :, ecc * 512:(ecc + 2) * 512])

        def push_wout(qc):
            for st in range(4 * qc, 4 * qc + 4):
                for half in range(2):
                    for i in range(4):
                        wout_q.append((st, half, i))

        def s1_emit_block(emit_trans=True):
            st = s1["st"]
            if st >= ST:
                return
            # only emit transposes whose postproc chain has had a full tile
            # of PE time to complete
            if emit_trans:
                for _ in range(2):
                    if trans_q and trans_q[0][0] <= st - 2:
                        emit_trans_item()
            if s1["blk"] == 0:
                s1["ts"] = stage1_alloc(st)
            b = s1["blk"]
            stage1_mm_block(s1["ts"], 4 * b, 4 * b + 4)
            if st == ST - 1 and 4 * b + 4 == DT:
                wo_dma()
            s1["blk"] += 1
            if s1["blk"] == DT // 4:
                s1["blk"] = 0
                s1["st"] += 1
                ts = s1["ts"]
                qfin, kfin = stage1_postproc(st, ts[1], ts[2])
                stage1_transposes(st, qfin, kfin)
                if st % 4 == 3:
                    qc = (st - 3) // 4
                    pend_attn.append((qc, 0))
                    pend_attn.append((qc, 1))
                if st == ST - 1:
                    s1["s1_done"] = True
                    for qc in attn_done_qcs:
                        push_wout(qc)

        wo_hold = [10]  # filler slots to skip while wo weights are in flight

        def emit_filler(n_s1, n_wout):
            while n_s1 > 0 and s1["st"] < ST:
                s1_emit_block(emit_trans=False)
                n_s1 -= 1
            if s1["st"] >= ST:
                for _ in range(min(2, len(trans_q))):
                    emit_trans_item()
                if wo_hold[0] > 0:
                    wo_hold[0] -= 1
                    return
            while n_wout > 0 and wout_q:
                emit_wout_unit()
                n_wout -= 1

        def run_attn_hp(qc, hp):
                while trans_q and trans_q[0][0] <= 4 * qc + 3:
                    emit_trans_item()
                if hp == 0 and s1["st"] < ST:
                    # a couple of stage-1 blocks of PE time for the fresh qT
                    # copies to land before the first score reads them
                    emit_filler(2, 0)
                hh0 = 2 * hp
                pv0 = psum.tile([128, 512], f32, tag="pa", bufs=4, name=f"pv0_{qc}_{hp}")
                pv1 = psum.tile([128, 512], f32, tag="pa", bufs=4, name=f"pv1_{qc}_{hp}")
                pvs = [pv0, pv1]
                exsum = [spool.tile([128, 512], f32, tag="exs", bufs=2,
                                    name=f"exsum_{qc}_{hp}_{hi}") for hi in range(2)]
                exsum_b = [spool.tile([128, 512], bf16, tag="exsb", bufs=2,
                                      name=f"exsumb_{qc}_{hp}_{hi}") for hi in range(2)]
                n_kt = 4 * qc + 4
                for kt in range(n_kt):
                    for _ in range(2):
                        if trans_q and (s1["st"] >= ST
                                        or trans_q[0][0] <= s1["st"] - 2):
                            emit_trans_item()
                    j = kt - 4 * qc
                    off = 0 if j < 0 else 128 * j
                    exs = []
                    for hi in range(2):
                        h = hh0 + hi
                        sc_ps = psum.tile([128, 512], f32, tag="pb", bufs=2,
                                          name=f"sc_{qc}_{hp}_{kt}_{hi}")
                        nc.tensor.matmul(
                            sc_ps[:, off:512],
                            kT_sb[:, kt * 128:(kt + 1) * 128],
                            qT_sb[:, h, qc * 512 + off:(qc + 1) * 512],
                            start=True, stop=True)
                        ex = epool.tile([128, 512], bf16, name=f"ex_{hi}")
                        nc.scalar.activation(out=ex[:, off:512],
                                             in_=sc_ps[:, off:512], func=AF.Exp)
                        if j >= 0:
                            nc.vector.tensor_mul(ex[:, off:off + 128],
                                                 ex[:, off:off + 128], mask_sb)
                        exs.append(ex)
                    # denominator partial sums on the DVE (keeps them off
                    # the PE); the diagonal-tile prefix copy goes to the Pool
                    for hi in range(2):
                        if kt == 0:
                            nc.vector.tensor_copy(out=exsum[hi][:, off:512],
                                                  in_=exs[hi][:, off:512])
                        elif kt < n_kt - 1:
                            nc.vector.tensor_add(exsum[hi][:, off:512],
                                                 exsum[hi][:, off:512],
                                                 exs[hi][:, off:512])
                        else:
                            if off > 0:
                                nc.vector.tensor_copy(out=exsum_b[hi][:, 0:off],
                                                      in_=exsum[hi][:, 0:off])
                            nc.vector.tensor_add(exsum_b[hi][:, off:512],
                                                 exsum[hi][:, off:512],
                                                 exs[hi][:, off:512])
                    # PE filler between scores and PV hides exp latency:
                    # stage-1 blocks while they last, out-proj units after
                    emit_filler(2, 1)
                    for hi in range(2):
                        nc.tensor.matmul(pvs[hi][:, off:512], v_sb[:, kt, :],
                                         exs[hi][:, off:512],
                                         start=(kt == 0), stop=(kt == n_kt - 1))
                # PE filler so the denominators' exp/add chain can finish
                emit_filler(1, 1)
                for hi in range(2):
                    h = hh0 + hi
                    den_ps = psum.tile([128, 512], f32, tag="pb", bufs=2,
                                       name=f"den_{qc}_{hp}_{hi}")
                    nc.tensor.matmul(den_ps, onesm_sb, exsum_b[hi],
                                     start=True, stop=True)
                    rden = scratch.tile([128, 512], f32, tag="rden")
                    nc.vector.reciprocal_approx_fast(out=rden, in_=den_ps)
                    nc.vector.tensor_mul(oT_sb[:, h, qc * 512:(qc + 1) * 512],
                                         pvs[hi], rden)


        while pend_attn or not s1["s1_done"] or wout_q:
            if pend_attn:
                qc, hp = pend_attn.pop(0)
                run_attn_hp(qc, hp)
                if hp == 1:
                    attn_done_qcs.append(qc)
                    if s1["s1_done"]:
                        push_wout(qc)
            elif not s1["s1_done"]:
                s1_emit_block()
            else:
                emit_wout_unit(tail=True)

    nc.compile()
    return nc


def _host_prep(x, w_qkv, w_out, q_ln_w, k_ln_w):
    """Build per-core input maps (host-side shard + transform)."""
    import ml_dtypes
    bf16 = ml_dtypes.bfloat16

    x2 = np.asarray(x, np.float32).reshape(S, D)
    # x tiles [st, d_local, d_tile, s_local] so each s-tile DMA is contiguous
    xt = np.ascontiguousarray(
        x2.reshape(ST, 128, DT, 128).transpose(0, 3, 2, 1)).astype(bf16)

    # rope tables (duplicated cos / sign-baked sin, interleaved layout)
    freqs = 1.0 / (THETA ** (np.arange(0, HD, 2, dtype=np.float64) / HD))
    ang = np.arange(S, dtype=np.float64)[:, None] * freqs[None, :]
    cos = np.cos(ang).astype(np.float32)
    sin = np.sin(ang).astype(np.float32)
    ccd = np.repeat(cos, 2, axis=1).astype(np.float16)    # [S, 128]
    ssd = np.stack([-sin, sin], axis=-1).reshape(S, HD).astype(np.float16)

    kq = np.arange(128)
    dmask = (kq[:, None] <= kq[None, :]).astype(bf16)     # [k, q]
    ident = np.eye(128, dtype=bf16)

    wq = np.asarray(w_qkv, np.float32)
    wo = np.asarray(w_out, np.float32)
    qw = np.asarray(q_ln_w, np.float32)
    kw = np.asarray(k_ln_w, np.float32)

    in_maps = []
    for g in range(N_CORES):
        wq_g = wq[512 * g:512 * (g + 1), :].reshape(HG, HD, D) * qw[None, :, None]
        wk_g = wq[D + 128 * g:D + 128 * (g + 1), :] * kw[:, None]
        wv_g = wq[D + G * HD + 128 * g:D + G * HD + 128 * (g + 1), :]
        wqkv_g = np.concatenate([wq_g.reshape(512, D), wk_g, wv_g], axis=0)
        wqkvT_g = np.ascontiguousarray(wqkv_g.T).astype(bf16)     # [D, 768]
        woT_g = np.ascontiguousarray(wo[:, 512 * g:512 * (g + 1)].T).astype(bf16)
        in_maps.append({
            "xt": xt,
            "wqkvT": wqkvT_g,
            "woT": woT_g,
            "ccd": ccd,
            "ssd": ssd,
            "dmask": dmask,
            "ident": ident,
        })
    return in_maps


_CACHE = {}


def _get_compiled():
    if "nc" not in _CACHE:
        _ensure_ntff_hook()
        _CACHE["nc"] = _build_nc()
    return _CACHE["nc"]


def run_sharded(x, w_qkv, w_out, q_ln_w, k_ln_w, trace=False):
    from concourse.bass_utils import run_bass_kernel_spmd
    nc = _get_compiled()
    in_maps = _host_prep(x, w_qkv, w_out, q_ln_w, k_ln_w)
    res = run_bass_kernel_spmd(nc, in_maps, core_ids=list(range(N_CORES)),
                               trace=trace)
    acc = np.zeros((S, D), np.float32)
    for i in range(N_CORES):
        acc += np.asarray(res.results[i]["out"], np.float32)
    return acc.reshape(1, S, D), res


def kernel(x, w_qkv, w_out, q_ln_w, k_ln_w):
    out, _ = run_sharded(x, w_qkv, w_out, q_ln_w, k_ln_w, trace=False)
    return out
